# revision 16
# baseline (speedup 1.0000x reference)
"""Dynamic depthwise 3x3 conv (per-pixel weights) on 8 Trainium2 NeuronCores.

Problem:
  x:            [4, 64, 256, 256]  f32
  conv_weights: [4, 576, 256, 256] f32  (= [4, 64ch * 9tap, 256, 256])
  out[n,c,h,w] = sum_k w[n, c*9+k, h, w] * xpad[n, c, h+ki, w+kj],  k=(ki,kj) row-major

Sharding: pure data parallel over (batch n, H-half) -> 8 shards.

On-core layout: partition p = hb*64 + c (hb in {0,1} picks a 64-row block,
c the channel). Free dim holds (h, w) with x rows padded to 258 so all nine
taps are pure free-dim AP offsets (dh*258 + dw). Loop over h-tiles of Rh
rows; per tap k: DVE multiply w_k with the shifted x view and accumulate.

All inputs/outputs are repacked on the host into per-tile-contiguous
[T, 128, bytes] blocks so every DMA is one sequential HBM stream with one
large contiguous descriptor per partition (scattered-read DMA measured at
~13 GB/s/engine vs ~27 for sequential).
"""

import sys

sys.path.insert(0, "/opt/trn_rl_repo")

import numpy as np

import concourse.bass as bass
import concourse.bacc as bacc
import concourse.tile as tile
from concourse import mybir
from concourse.bass_utils import run_bass_kernel_spmd

N, C, H, W = 4, 64, 256, 256
KW = 3
NCORES = 8
HH = H // 2          # rows per core
RB = HH // 2         # rows per partition block (64)
Rh = 4               # rows per h-tile
T = RB // Rh         # h-tiles per core
Wp = W + 2           # padded row width
NXT = 4              # resident x tiles per core
XB = RB // NXT       # local output rows covered per x tile (16)
XR = XB + 2          # rows per resident x tile incl halo
XF = XR * Wp         # x tile free elems
WF = KW * KW * Rh * W
OF = Rh * W
F32 = mybir.dt.float32

_CACHE = {}


def _build():
    from segmac import get_segmac_op, window_ap

    op = get_segmac_op()
    nc = bacc.Bacc("TRN2", target_bir_lowering=False, debug=False, num_devices=NCORES)
    x_in = nc.dram_tensor("x", [NXT, 128, XF], F32, kind="ExternalInput")
    w_in = nc.dram_tensor("w", [T, 128, WF], F32, kind="ExternalInput")
    y_out = nc.dram_tensor("y", [T, 128, OF], F32, kind="ExternalOutput")

    with tile.TileContext(nc) as tc:
        with (
            tc.tile_pool(name="xp", bufs=1) as xpool,
            tc.tile_pool(name="wp", bufs=2) as wpool,
            tc.tile_pool(name="op", bufs=2) as opool,
            tc.tile_pool(name="pa", bufs=1) as papool,
            tc.tile_pool(name="pb", bufs=1) as pbpool,
        ):
            # x stays resident: NXT tiles, each covering XB output rows
            # (+2 halo rows) per partition block, loaded once.
            xtiles = []
            for s in range(NXT):
                xt = xpool.tile([128, XF], F32, tag=f"x{s}")
                nc.scalar.dma_start(out=xt[:], in_=x_in[s])
                xtiles.append(xt)

            for t in range(T):
                wt = wpool.tile([128, WF], F32)
                # 3 chunked loads (one per dh k-triplet) so the first MACs
                # can start before the whole tile lands.
                for dh in range(KW):
                    c0 = dh * 3 * Rh * W
                    nc.sync.dma_start(
                        out=wt[:, c0:c0 + 3 * Rh * W],
                        in_=w_in[t, :, c0:c0 + 3 * Rh * W],
                    )

                xt = xtiles[t * Rh // XB]
                rbase = t * Rh - (t * Rh // XB) * XB

                ot = opool.tile([128, OF], F32)
                pa = papool.tile([128, OF], F32)
                pb = pbpool.tile([128, OF], F32)
                # per (dh, row): 3-tap segmented MAC
                #   target[p, r*W + wd] = sum_dw w[(dh*3+dw), r, wd] * x[r+dh, wd+dw]
                for dh, tgt in ((0, ot), (1, pa), (2, pb)):
                    for r in range(Rh):
                        w_sl = wt[:, dh * 3 * Rh * W + r * W:
                                  dh * 3 * Rh * W + r * W + 2 * Rh * W + W]
                        xrow = rbase + r + dh
                        x_sl = xt[:, xrow * Wp:(xrow + 1) * Wp]
                        o_sl = tgt[:, r * W:(r + 1) * W]
                        nc.vector._custom_dve(
                            op,
                            out=window_ap(o_sl, [[1, W], [0, KW]]),
                            in0=window_ap(w_sl, [[1, W], [Rh * W, KW]]),
                            in1=window_ap(x_sl, [[1, W], [1, KW]]),
                        )
                nc.gpsimd.tensor_add(ot[:], ot[:], pa[:])
                nc.gpsimd.tensor_add(ot[:], ot[:], pb[:])

                nc.scalar.dma_start(out=y_out[t], in_=ot[:])
    nc.compile()
    return nc


def _get_nc():
    if "nc" not in _CACHE:
        _CACHE["nc"] = _build()
    return _CACHE["nc"]


def _pack_core(xp_n: np.ndarray, w5_n: np.ndarray, hf: int):
    """Repack one core's shard into per-tile-contiguous DMA blocks.

    xp_n: [C, H+2, Wp] host-padded x for batch n; w5_n: [C, 9, H, W].
    Returns x_blocks [T, 128, XF], w_blocks [T, 128, WF].
    """
    xc = xp_n[:, hf * HH:hf * HH + HH + 2, :]          # [C, HH+2, Wp]
    wc = w5_n[:, :, hf * HH:(hf + 1) * HH, :]          # [C, 9, HH, W]

    xb = np.empty((NXT, 2, C, XR, Wp), dtype=np.float32)  # [xtile, hb, C, XR, Wp]
    for s in range(NXT):
        for hb in range(2):
            r0 = hb * RB + s * XB
            xb[s, hb] = xc[:, r0:r0 + XR, :]
    # w: h = (hb, t, h_sub) -> [T, hb, C, 9, Rh, W]
    wb = (
        wc.reshape(C, KW * KW, 2, T, Rh, W)
        .transpose(3, 2, 0, 1, 4, 5)
        .reshape(T, 128, WF)
    )
    return xb.reshape(NXT, 128, XF), np.ascontiguousarray(wb)


def _make_in_maps(x: np.ndarray, conv_weights: np.ndarray):
    x = np.asarray(x, dtype=np.float32)
    w5 = np.asarray(conv_weights, dtype=np.float32).reshape(N, C, KW * KW, H, W)
    xp = np.pad(x, ((0, 0), (0, 0), (1, 1), (1, 1)))

    in_maps = []
    for i in range(NCORES):
        n, hf = divmod(i, 2)
        xb, wb = _pack_core(xp[n], w5[n], hf)
        in_maps.append({"x": xb, "w": wb})
    return in_maps


def kernel(x: np.ndarray, conv_weights: np.ndarray) -> np.ndarray:
    nc = _get_nc()
    in_maps = _make_in_maps(x, conv_weights)
    res = run_bass_kernel_spmd(nc, in_maps, list(range(NCORES)))
    out = np.empty((N, C, H, W), dtype=np.float32)
    for i in range(NCORES):
        n, hf = divmod(i, 2)
        yb = res.results[i]["y"].reshape(T, 2, C, Rh, W)
        # invert: out rows h = hf*HH + hb*RB + t*Rh + h_sub
        oc = yb.transpose(2, 1, 0, 3, 4).reshape(C, HH, W)
        out[n, :, hf * HH:(hf + 1) * HH, :] = oc
    return out


# revision 17
# speedup vs baseline: 1.0276x; 1.0276x over previous
"""Dynamic depthwise 3x3 conv (per-pixel weights) on 8 Trainium2 NeuronCores.

Problem:
  x:            [4, 64, 256, 256]  f32
  conv_weights: [4, 576, 256, 256] f32  (= [4, 64ch * 9tap, 256, 256])
  out[n,c,h,w] = sum_k w[n, c*9+k, h, w] * xpad[n, c, h+ki, w+kj],  k=(ki,kj) row-major

Sharding: pure data parallel over (batch n, H-half) -> 8 shards.

On-core layout: partition p = hb*64 + c (hb in {0,1} picks a 64-row block,
c the channel). Free dim holds (h, w) with x rows padded to 258 so all nine
taps are pure free-dim AP offsets (dh*258 + dw). Loop over h-tiles of Rh
rows; per tap k: DVE multiply w_k with the shifted x view and accumulate.

All inputs/outputs are repacked on the host into per-tile-contiguous
[T, 128, bytes] blocks so every DMA is one sequential HBM stream with one
large contiguous descriptor per partition (scattered-read DMA measured at
~13 GB/s/engine vs ~27 for sequential).
"""

import sys

sys.path.insert(0, "/opt/trn_rl_repo")

import numpy as np

import concourse.bass as bass
import concourse.bacc as bacc
import concourse.tile as tile
from concourse import mybir
from concourse.bass_utils import run_bass_kernel_spmd

N, C, H, W = 4, 64, 256, 256
KW = 3
NCORES = 8
HH = H // 2          # rows per core
RB = HH // 2         # rows per partition block (64)
Rh = 4               # rows per h-tile
T = RB // Rh         # h-tiles per core
Wp = W + 2           # padded row width
NXT = 4              # resident x tiles per core
XB = RB // NXT       # local output rows covered per x tile (16)
XR = XB + 2          # rows per resident x tile incl halo
XF = XR * Wp         # x tile free elems
WF = KW * KW * Rh * W
OF = Rh * W
F32 = mybir.dt.float32

_CACHE = {}


def _build():
    from segmac import get_segmac_op, window_ap

    op = get_segmac_op()
    nc = bacc.Bacc("TRN2", target_bir_lowering=False, debug=False, num_devices=NCORES)
    x_in = nc.dram_tensor("x", [NXT, 128, XF], F32, kind="ExternalInput")
    w_in = nc.dram_tensor("w", [T, 128, WF], F32, kind="ExternalInput")
    y_out = nc.dram_tensor("y", [T, 128, OF], F32, kind="ExternalOutput")

    with tile.TileContext(nc) as tc:
        with (
            tc.tile_pool(name="xp", bufs=1) as xpool,
            tc.tile_pool(name="wp", bufs=2) as wpool,
            tc.tile_pool(name="op", bufs=2) as opool,
            tc.tile_pool(name="pa", bufs=2) as papool,
            tc.tile_pool(name="pb", bufs=2) as pbpool,
        ):
            # x stays resident: NXT tiles, each covering XB output rows
            # (+2 halo rows) per partition block, loaded once.
            xtiles = []
            for s in range(NXT):
                xt = xpool.tile([128, XF], F32, tag=f"x{s}")
                nc.scalar.dma_start(out=xt[:], in_=x_in[s])
                xtiles.append(xt)

            for t in range(T):
                wt = wpool.tile([128, WF], F32)
                # 3 chunked loads (one per dh k-triplet) so the first MACs
                # can start before the whole tile lands.
                for dh in range(KW):
                    c0 = dh * 3 * Rh * W
                    nc.sync.dma_start(
                        out=wt[:, c0:c0 + 3 * Rh * W],
                        in_=w_in[t, :, c0:c0 + 3 * Rh * W],
                    )

                xt = xtiles[t * Rh // XB]
                rbase = t * Rh - (t * Rh // XB) * XB

                ot = opool.tile([128, OF], F32)
                pa = papool.tile([128, OF], F32)
                pb = pbpool.tile([128, OF], F32)
                # per (dh, row): 3-tap segmented MAC
                #   target[p, r*W + wd] = sum_dw w[(dh*3+dw), r, wd] * x[r+dh, wd+dw]
                for dh, tgt in ((0, ot), (1, pa), (2, pb)):
                    for r in range(Rh):
                        w_sl = wt[:, dh * 3 * Rh * W + r * W:
                                  dh * 3 * Rh * W + r * W + 2 * Rh * W + W]
                        xrow = rbase + r + dh
                        x_sl = xt[:, xrow * Wp:(xrow + 1) * Wp]
                        o_sl = tgt[:, r * W:(r + 1) * W]
                        nc.vector._custom_dve(
                            op,
                            out=window_ap(o_sl, [[1, W], [0, KW]]),
                            in0=window_ap(w_sl, [[1, W], [Rh * W, KW]]),
                            in1=window_ap(x_sl, [[1, W], [1, KW]]),
                        )
                nc.gpsimd.tensor_add(ot[:], ot[:], pa[:])
                nc.gpsimd.tensor_add(ot[:], ot[:], pb[:])

                nc.scalar.dma_start(out=y_out[t], in_=ot[:])
    nc.compile()
    return nc


def _get_nc():
    if "nc" not in _CACHE:
        _CACHE["nc"] = _build()
    return _CACHE["nc"]


def _pack_core(xp_n: np.ndarray, w5_n: np.ndarray, hf: int):
    """Repack one core's shard into per-tile-contiguous DMA blocks.

    xp_n: [C, H+2, Wp] host-padded x for batch n; w5_n: [C, 9, H, W].
    Returns x_blocks [T, 128, XF], w_blocks [T, 128, WF].
    """
    xc = xp_n[:, hf * HH:hf * HH + HH + 2, :]          # [C, HH+2, Wp]
    wc = w5_n[:, :, hf * HH:(hf + 1) * HH, :]          # [C, 9, HH, W]

    xb = np.empty((NXT, 2, C, XR, Wp), dtype=np.float32)  # [xtile, hb, C, XR, Wp]
    for s in range(NXT):
        for hb in range(2):
            r0 = hb * RB + s * XB
            xb[s, hb] = xc[:, r0:r0 + XR, :]
    # w: h = (hb, t, h_sub) -> [T, hb, C, 9, Rh, W]
    wb = (
        wc.reshape(C, KW * KW, 2, T, Rh, W)
        .transpose(3, 2, 0, 1, 4, 5)
        .reshape(T, 128, WF)
    )
    return xb.reshape(NXT, 128, XF), np.ascontiguousarray(wb)


def _make_in_maps(x: np.ndarray, conv_weights: np.ndarray):
    x = np.asarray(x, dtype=np.float32)
    w5 = np.asarray(conv_weights, dtype=np.float32).reshape(N, C, KW * KW, H, W)
    xp = np.pad(x, ((0, 0), (0, 0), (1, 1), (1, 1)))

    in_maps = []
    for i in range(NCORES):
        n, hf = divmod(i, 2)
        xb, wb = _pack_core(xp[n], w5[n], hf)
        in_maps.append({"x": xb, "w": wb})
    return in_maps


def kernel(x: np.ndarray, conv_weights: np.ndarray) -> np.ndarray:
    nc = _get_nc()
    in_maps = _make_in_maps(x, conv_weights)
    res = run_bass_kernel_spmd(nc, in_maps, list(range(NCORES)))
    out = np.empty((N, C, H, W), dtype=np.float32)
    for i in range(NCORES):
        n, hf = divmod(i, 2)
        yb = res.results[i]["y"].reshape(T, 2, C, Rh, W)
        # invert: out rows h = hf*HH + hb*RB + t*Rh + h_sub
        oc = yb.transpose(2, 1, 0, 3, 4).reshape(C, HH, W)
        out[n, :, hf * HH:(hf + 1) * HH, :] = oc
    return out
